# revision 2
# baseline (speedup 1.0000x reference)
"""nn_BinaryLinear TRN2 kernel: out = x @ sign(weight).T + sign(bias).

Host pre-transposes/casts inputs so the device runs a pure matmul stream
(no on-device transposes). Sharding: batch 4-way x out-dim 2-way; each core
computes a [2048, 2048] block of the [8192, 4096] output.

Modes:
  fp16   - x,W in fp16; 2048 MMs/core @ N=512 (PE streaming floor ~437us)
  hybN   - hybrid: first N K-blocks (of 32) as packed e4m3 DoubleRow pairs
           (2 K-blocks per MM -> half the MMs for that span; x quantization
           error 2.64e-2 * sqrt(N/32)), remaining 32-N blocks in fp16.
           hyb0 == fp16 path shape; hyb32 = pure packed fp8.

Bias is added on host (exact f32). Output written as fp16, upcast on host.
"""

from contextlib import ExitStack

import numpy as np
import ml_dtypes

import concourse.bass as bass
import concourse.tile as tile
from concourse import bacc, mybir
from concourse.bass_utils import run_bass_kernel_spmd

P = 128
F32 = mybir.dt.float32
FP16 = mybir.dt.float16
FP8 = mybir.dt.float8e4
F8NP = ml_dtypes.float8_e4m3
DR = mybir.MatmulPerfMode.DoubleRow

B, K, O = 8192, 4096, 4096
BSHARD, OSHARD = 4, 2
Bs, Os = B // BSHARD, O // OSHARD  # 2048, 2048
KT = K // P  # 32
MT = Bs // P  # 16
NFREE = 512
NT = Os // NFREE  # 4
FILL_M = 2

MODE = "hyb14"


def _parse_mode(mode):
    if mode == "fp16":
        return 0
    if mode.startswith("hyb"):
        fp8_kt = int(mode[3:])
        assert fp8_kt % 2 == 0 and 0 <= fp8_kt <= KT
        return fp8_kt
    raise ValueError(mode)


def _build(mode):
    fp8_kt = _parse_mode(mode)
    ktp8 = fp8_kt // 2  # packed fp8 pair-chunks
    kt16 = KT - fp8_kt  # fp16 chunks

    nc = bacc.Bacc("TRN2", target_bir_lowering=False, debug=False)
    xprep8 = wprep8 = xprep16 = wprep16 = None
    if ktp8:
        xprep8 = nc.dram_tensor(
            "xprep8", [MT, P, ktp8, 2, P], FP8, kind="ExternalInput"
        ).ap()
        wprep8 = nc.dram_tensor(
            "wprep8", [P, ktp8, 2, Os], FP8, kind="ExternalInput"
        ).ap()
    if kt16:
        xprep16 = nc.dram_tensor(
            "xprep16", [MT, P, kt16, P], FP16, kind="ExternalInput"
        ).ap()
        wprep16 = nc.dram_tensor(
            "wprep16", [P, kt16, Os], FP16, kind="ExternalInput"
        ).ap()

    out = nc.dram_tensor("out", [Bs, Os], FP16, kind="ExternalOutput").ap()
    out_rows = out.rearrange("(m p) o -> p m o", p=P)

    # Chunk order: grouped by dtype (measured: alternating dtypes costs
    # ~+8ns/MM and DR LDWEIGHTS never hides under fp16 streams anyway).
    # Chunk id: ("f16", t) or ("f8", j).
    chunk_order = [("f8", j) for j in range(ktp8)] + [("f16", t) for t in range(kt16)]
    n_chunks = len(chunk_order)

    with tile.TileContext(nc) as tc, ExitStack() as ctx:
        wpool = ctx.enter_context(tc.tile_pool(name="w", bufs=1))
        xpool = ctx.enter_context(tc.tile_pool(name="x", bufs=3))
        opool = ctx.enter_context(tc.tile_pool(name="o", bufs=6))
        psum = ctx.enter_context(tc.tile_pool(name="psum", bufs=8, space="PSUM"))

        w8_sb = wpool.tile([P, ktp8, 2, Os], FP8, name="w8_sb") if ktp8 else None
        w16_sb = wpool.tile([P, kt16, Os], FP16, name="w16_sb") if kt16 else None

        def stage_x(m):
            # x on the gpsimd DMA queue so it never queues behind the 12-16MB
            # W stream (the first fill matmuls need x tiles within ~5us)
            tiles = {}
            if ktp8:
                t8 = xpool.tile([P, ktp8, 2, P], FP8, tag="x8", name="t8")
                nc.gpsimd.dma_start(out=t8[:], in_=xprep8[m])
                tiles["f8"] = t8
            if kt16:
                t16 = xpool.tile([P, kt16, P], FP16, tag="x16", name="t16")
                nc.gpsimd.dma_start(out=t16[:], in_=xprep16[m])
                tiles["f16"] = t16
            return tiles

        def load_w(ci_range):
            # W streams in chunk (consumption) order across two DMA queues;
            # gpsimd stays free for x prefetches (a W backlog there delays
            # steady-state x tiles and stalls the PE)
            engs = [nc.sync, nc.scalar]
            for ci in ci_range:
                kind, idx = chunk_order[ci]
                eng = engs[ci % 2]
                if kind == "f8":
                    eng.dma_start(out=w8_sb[:, idx], in_=wprep8[:, idx])
                else:
                    eng.dma_start(out=w16_sb[:, idx], in_=wprep16[:, idx])

        def mm(ps, xt, c, n, start, stop):
            ns = slice(n * NFREE, (n + 1) * NFREE)
            kind, idx = chunk_order[c]
            if kind == "f8":
                nc.tensor.matmul(
                    ps[:],
                    xt["f8"][:, idx],
                    w8_sb[:, idx, :, ns],
                    start=start,
                    stop=stop,
                    perf_mode=DR,
                )
            else:
                nc.tensor.matmul(
                    ps[:],
                    xt["f16"][:, idx],
                    w16_sb[:, idx, ns],
                    start=start,
                    stop=stop,
                )

        def evict(m, n, ps):
            o16 = opool.tile([P, NFREE], FP16, tag="o16", name="o16")
            nc.vector.tensor_copy(out=o16[:], in_=ps[:])
            nc.sync.dma_start(out_rows[:, m, n * NFREE : (n + 1) * NFREE], o16[:])

        # fill: process FILL_M m-tiles chunk-major so the PE consumes W chunks
        # as they stream in (W load ~40-55us; one m-tile can't cover it).
        # Issue order puts the first matmul's inputs (x0 + W chunk 0) at the
        # head of every queue so it can start as early as possible.
        xts = {0: stage_x(0)}
        load_w(range(1))
        for m in range(1, FILL_M):
            xts[m] = stage_x(m)
        load_w(range(1, n_chunks))
        fill_ps = {
            (m, n): psum.tile([P, NFREE], F32, tag="pm", name=f"pmf_{m}_{n}")
            for m in range(FILL_M)
            for n in range(NT)
        }
        for c in range(n_chunks):
            for m in range(FILL_M):
                for n in range(NT):
                    mm(fill_ps[m, n], xts[m], c, n, c == 0, c == n_chunks - 1)
        if FILL_M < MT:
            xts[FILL_M] = stage_x(FILL_M)
        for m in range(FILL_M):
            for n in range(NT):
                evict(m, n, fill_ps[m, n])

        # steady state
        for m in range(FILL_M, MT):
            if m + 1 < MT:
                xts[m + 1] = stage_x(m + 1)
            xt = xts.pop(m)
            # n-outer: each (m, n) chain runs its interleaved chunk sequence
            # back-to-back, and chain n's eviction overlaps chain n+1
            for n in range(NT):
                pm = psum.tile([P, NFREE], F32, tag="pm", name=f"pm_{m}_{n}")
                for c in range(n_chunks):
                    mm(pm, xt, c, n, c == 0, c == n_chunks - 1)
                evict(m, n, pm)

    nc.compile()
    return nc


_NC_CACHE = {}


def _get_nc(mode):
    if mode not in _NC_CACHE:
        _NC_CACHE[mode] = _build(mode)
    return _NC_CACHE[mode]


def _prep_inputs(mode, x, weight):
    """Returns per-core input maps."""
    fp8_kt = _parse_mode(mode)
    ktp8 = fp8_kt // 2
    kt16 = KT - fp8_kt
    ksplit = fp8_kt * P

    in_maps = [{} for _ in range(8)]
    if ktp8:
        x8 = x[:, :ksplit].astype(F8NP)
        w8 = weight[:, :ksplit].astype(F8NP)
        for c in range(8):
            bi, oj = divmod(c, OSHARD)
            a = x8[bi * Bs : (bi + 1) * Bs].reshape(MT, P, ktp8, 2, P)
            in_maps[c]["xprep8"] = np.ascontiguousarray(a.transpose(0, 4, 2, 3, 1))
            wr = w8[oj * Os : (oj + 1) * Os].reshape(Os, ktp8, 2, P)
            in_maps[c]["wprep8"] = np.ascontiguousarray(wr.transpose(3, 1, 2, 0))
    if kt16:
        x16 = x[:, ksplit:].astype(np.float16)
        w16 = weight[:, ksplit:].astype(np.float16)
        for c in range(8):
            bi, oj = divmod(c, OSHARD)
            a = x16[bi * Bs : (bi + 1) * Bs].reshape(MT, P, kt16, P)
            in_maps[c]["xprep16"] = np.ascontiguousarray(a.transpose(0, 3, 2, 1))
            wr = w16[oj * Os : (oj + 1) * Os].reshape(Os, kt16, P)
            in_maps[c]["wprep16"] = np.ascontiguousarray(wr.transpose(2, 1, 0))
    return in_maps


def kernel(x, weight, bias, _trace=False, _mode=None, **_kw):
    mode = _mode or MODE
    x = np.asarray(x, dtype=np.float32)
    weight = np.asarray(weight, dtype=np.float32)
    bias = np.asarray(bias, dtype=np.float32)
    bias_s = np.sign(np.where(bias == 0, 1e-10, bias)).astype(np.float32)

    nc = _get_nc(mode)
    in_maps = _prep_inputs(mode, x, weight)
    res = run_bass_kernel_spmd(nc, in_maps, core_ids=list(range(8)), trace=_trace)

    out = np.empty((B, O), dtype=np.float32)
    for c in range(8):
        bi, oj = divmod(c, OSHARD)
        out[bi * Bs : (bi + 1) * Bs, oj * Os : (oj + 1) * Os] = (
            res.results[c]["out"].astype(np.float32) + bias_s[oj * Os : (oj + 1) * Os]
        )
    if _trace:
        kernel.last_results = res
    return out


# revision 3
# speedup vs baseline: 1.0554x; 1.0554x over previous
"""nn_BinaryLinear TRN2 kernel: out = x @ sign(weight).T + sign(bias).

Host pre-transposes/casts inputs so the device runs a pure matmul stream
(no on-device transposes). Sharding: batch 4-way x out-dim 2-way; each core
computes a [2048, 2048] block of the [8192, 4096] output.

Modes:
  fp16   - x,W in fp16; 2048 MMs/core @ N=512 (PE streaming floor ~437us)
  hybN   - hybrid: first N K-blocks (of 32) as packed e4m3 DoubleRow pairs
           (2 K-blocks per MM -> half the MMs for that span; x quantization
           error 2.64e-2 * sqrt(N/32)), remaining 32-N blocks in fp16.
           hyb0 == fp16 path shape; hyb32 = pure packed fp8.

Bias is added on host (exact f32). Output written as fp16, upcast on host.
"""

from contextlib import ExitStack

import numpy as np
import ml_dtypes

import concourse.bass as bass
import concourse.tile as tile
from concourse import bacc, mybir
from concourse.bass_utils import run_bass_kernel_spmd

P = 128
F32 = mybir.dt.float32
FP16 = mybir.dt.float16
FP8 = mybir.dt.float8e4
F8NP = ml_dtypes.float8_e4m3
DR = mybir.MatmulPerfMode.DoubleRow

B, K, O = 8192, 4096, 4096
BSHARD, OSHARD = 4, 2
Bs, Os = B // BSHARD, O // OSHARD  # 2048, 2048
KT = K // P  # 32
MT = Bs // P  # 16
NFREE = 512
NT = Os // NFREE  # 4
FILL_M = 2

MODE = "hyb16"


def _parse_mode(mode):
    if mode == "fp16":
        return 0
    if mode.startswith("hyb"):
        fp8_kt = int(mode[3:])
        assert fp8_kt % 2 == 0 and 0 <= fp8_kt <= KT
        return fp8_kt
    raise ValueError(mode)


def _build(mode):
    fp8_kt = _parse_mode(mode)
    ktp8 = fp8_kt // 2  # packed fp8 pair-chunks
    kt16 = KT - fp8_kt  # fp16 chunks

    nc = bacc.Bacc("TRN2", target_bir_lowering=False, debug=False)
    xprep8 = wprep8 = xprep16 = wprep16 = None
    if ktp8:
        xprep8 = nc.dram_tensor(
            "xprep8", [MT, P, ktp8, 2, P], FP8, kind="ExternalInput"
        ).ap()
        wprep8 = nc.dram_tensor(
            "wprep8", [P, ktp8, 2, Os], FP8, kind="ExternalInput"
        ).ap()
    if kt16:
        xprep16 = nc.dram_tensor(
            "xprep16", [MT, P, kt16, P], FP16, kind="ExternalInput"
        ).ap()
        wprep16 = nc.dram_tensor(
            "wprep16", [P, kt16, Os], FP16, kind="ExternalInput"
        ).ap()

    out = nc.dram_tensor("out", [Bs, Os], FP16, kind="ExternalOutput").ap()
    out_rows = out.rearrange("(m p) o -> p m o", p=P)

    # Chunk order: grouped by dtype (measured: alternating dtypes costs
    # ~+8ns/MM and DR LDWEIGHTS never hides under fp16 streams anyway).
    # Chunk id: ("f16", t) or ("f8", j).
    chunk_order = [("f8", j) for j in range(ktp8)] + [("f16", t) for t in range(kt16)]
    n_chunks = len(chunk_order)

    with tile.TileContext(nc) as tc, ExitStack() as ctx:
        wpool = ctx.enter_context(tc.tile_pool(name="w", bufs=1))
        xpool = ctx.enter_context(tc.tile_pool(name="x", bufs=3))
        opool = ctx.enter_context(tc.tile_pool(name="o", bufs=6))
        psum = ctx.enter_context(tc.tile_pool(name="psum", bufs=8, space="PSUM"))

        w8_sb = wpool.tile([P, ktp8, 2, Os], FP8, name="w8_sb") if ktp8 else None
        w16_sb = wpool.tile([P, kt16, Os], FP16, name="w16_sb") if kt16 else None

        def stage_x(m):
            # x on the gpsimd DMA queue so it never queues behind the 12-16MB
            # W stream (the first fill matmuls need x tiles within ~5us)
            tiles = {}
            if ktp8:
                t8 = xpool.tile([P, ktp8, 2, P], FP8, tag="x8", name="t8")
                nc.gpsimd.dma_start(out=t8[:], in_=xprep8[m])
                tiles["f8"] = t8
            if kt16:
                t16 = xpool.tile([P, kt16, P], FP16, tag="x16", name="t16")
                nc.gpsimd.dma_start(out=t16[:], in_=xprep16[m])
                tiles["f16"] = t16
            return tiles

        def load_w(ci_range):
            # W streams in chunk (consumption) order across two DMA queues;
            # gpsimd stays free for x prefetches (a W backlog there delays
            # steady-state x tiles and stalls the PE)
            engs = [nc.sync, nc.scalar]
            for ci in ci_range:
                kind, idx = chunk_order[ci]
                eng = engs[ci % 2]
                if kind == "f8":
                    eng.dma_start(out=w8_sb[:, idx], in_=wprep8[:, idx])
                else:
                    eng.dma_start(out=w16_sb[:, idx], in_=wprep16[:, idx])

        def mm(ps, xt, c, n, start, stop):
            ns = slice(n * NFREE, (n + 1) * NFREE)
            kind, idx = chunk_order[c]
            if kind == "f8":
                nc.tensor.matmul(
                    ps[:],
                    xt["f8"][:, idx],
                    w8_sb[:, idx, :, ns],
                    start=start,
                    stop=stop,
                    perf_mode=DR,
                )
            else:
                nc.tensor.matmul(
                    ps[:],
                    xt["f16"][:, idx],
                    w16_sb[:, idx, ns],
                    start=start,
                    stop=stop,
                )

        def evict(m, n, ps):
            o16 = opool.tile([P, NFREE], FP16, tag="o16", name="o16")
            nc.vector.tensor_copy(out=o16[:], in_=ps[:])
            nc.sync.dma_start(out_rows[:, m, n * NFREE : (n + 1) * NFREE], o16[:])

        # fill: process FILL_M m-tiles chunk-major so the PE consumes W chunks
        # as they stream in (W load ~40-55us; one m-tile can't cover it).
        # Issue order puts the first matmul's inputs (x0 + W chunk 0) at the
        # head of every queue so it can start as early as possible.
        xts = {0: stage_x(0)}
        load_w(range(1))
        for m in range(1, FILL_M):
            xts[m] = stage_x(m)
        load_w(range(1, n_chunks))
        fill_ps = {
            (m, n): psum.tile([P, NFREE], F32, tag="pm", name=f"pmf_{m}_{n}")
            for m in range(FILL_M)
            for n in range(NT)
        }
        for c in range(n_chunks):
            for m in range(FILL_M):
                for n in range(NT):
                    mm(fill_ps[m, n], xts[m], c, n, c == 0, c == n_chunks - 1)
        if FILL_M < MT:
            xts[FILL_M] = stage_x(FILL_M)
        for m in range(FILL_M):
            for n in range(NT):
                evict(m, n, fill_ps[m, n])

        # steady state
        for m in range(FILL_M, MT):
            if m + 1 < MT:
                xts[m + 1] = stage_x(m + 1)
            xt = xts.pop(m)
            # n-outer: each (m, n) chain runs its interleaved chunk sequence
            # back-to-back, and chain n's eviction overlaps chain n+1
            for n in range(NT):
                pm = psum.tile([P, NFREE], F32, tag="pm", name=f"pm_{m}_{n}")
                for c in range(n_chunks):
                    mm(pm, xt, c, n, c == 0, c == n_chunks - 1)
                evict(m, n, pm)

    nc.compile()
    return nc


_NC_CACHE = {}


def _get_nc(mode):
    if mode not in _NC_CACHE:
        _NC_CACHE[mode] = _build(mode)
    return _NC_CACHE[mode]


def _prep_inputs(mode, x, weight):
    """Returns per-core input maps."""
    fp8_kt = _parse_mode(mode)
    ktp8 = fp8_kt // 2
    kt16 = KT - fp8_kt
    ksplit = fp8_kt * P

    in_maps = [{} for _ in range(8)]
    if ktp8:
        x8 = x[:, :ksplit].astype(F8NP)
        w8 = weight[:, :ksplit].astype(F8NP)
        for c in range(8):
            bi, oj = divmod(c, OSHARD)
            a = x8[bi * Bs : (bi + 1) * Bs].reshape(MT, P, ktp8, 2, P)
            in_maps[c]["xprep8"] = np.ascontiguousarray(a.transpose(0, 4, 2, 3, 1))
            wr = w8[oj * Os : (oj + 1) * Os].reshape(Os, ktp8, 2, P)
            in_maps[c]["wprep8"] = np.ascontiguousarray(wr.transpose(3, 1, 2, 0))
    if kt16:
        x16 = x[:, ksplit:].astype(np.float16)
        w16 = weight[:, ksplit:].astype(np.float16)
        for c in range(8):
            bi, oj = divmod(c, OSHARD)
            a = x16[bi * Bs : (bi + 1) * Bs].reshape(MT, P, kt16, P)
            in_maps[c]["xprep16"] = np.ascontiguousarray(a.transpose(0, 3, 2, 1))
            wr = w16[oj * Os : (oj + 1) * Os].reshape(Os, kt16, P)
            in_maps[c]["wprep16"] = np.ascontiguousarray(wr.transpose(2, 1, 0))
    return in_maps


def kernel(x, weight, bias, _trace=False, _mode=None, **_kw):
    mode = _mode or MODE
    x = np.asarray(x, dtype=np.float32)
    weight = np.asarray(weight, dtype=np.float32)
    bias = np.asarray(bias, dtype=np.float32)
    bias_s = np.sign(np.where(bias == 0, 1e-10, bias)).astype(np.float32)

    nc = _get_nc(mode)
    in_maps = _prep_inputs(mode, x, weight)
    res = run_bass_kernel_spmd(nc, in_maps, core_ids=list(range(8)), trace=_trace)

    out = np.empty((B, O), dtype=np.float32)
    for c in range(8):
        bi, oj = divmod(c, OSHARD)
        out[bi * Bs : (bi + 1) * Bs, oj * Os : (oj + 1) * Os] = (
            res.results[c]["out"].astype(np.float32) + bias_s[oj * Os : (oj + 1) * Os]
        )
    if _trace:
        kernel.last_results = res
    return out
